# revision 26
# baseline (speedup 1.0000x reference)
"""Bass/Trainium2 kernel for nn_BoxFilter: 9x9 circular box-mean over
(8, 3, 1024, 1024) f32, data-parallel across 8 NeuronCores (1 image/core).

bf16 I/O (rel-err gate is 2e-2; end-to-end bf16 keeps ~4e-3). Per 128-row
input block (120 output rows):
  - vertical pass: ones-band matmul on PE -> PSUM f32 (exact 9-row sums)
  - ACT evicts PSUM with x(1/81) scale + downcast into a wrap-padded bf16
    segment [9 zeros | wrap 4 | 1024 | wrap 4]; the two 4-col wraps are one
    ACT op with a 2-group negative-stride access pattern.
  - horizontal pass: running-box DVE scan state[t] += u[t+9] - u[t], the
    bottleneck engine (~2.17 ns/col, dtype-independent, DVE-only opcode;
    GpSimd co-running slows DVE more than it helps - measured). Two 120-row
    blocks concatenate into one 2082-col buffer swept by a single scan; the
    9 zero warmup cols per segment absorb window contamination so segments
    chain with no initial-state handoff.
  - the zero warmup cols are memset only on each u-buffer's first rotation
    (evict/wraps never touch them), so steady-state scans depend on ACT only.
  - loads + late stores on Sync ring, other stores on GpSimd ring; pair
    loads and stores are single 2D DMAs (~0.5 MB).
"""

import numpy as np
import ml_dtypes

import concourse.bacc as bacc
import concourse.mybir as mybir
import concourse.tile as tile
from concourse.ap import AP
from concourse.bass_utils import run_bass_kernel_spmd

B, C, H, W = 8, 3, 1024, 1024
R = 4            # filter radius
WIN = 2 * R + 1  # 9
AREA = WIN * WIN
MBLK = 120       # output rows per 128-row input block
SEG = WIN + W + 2 * R  # 1041: one block's scan segment
MT = H - 8 * MBLK  # 64 tail output rows
KT = MT + 2 * R    # 72 tail input rows
UBUFS = 4          # u-pool rotation depth (zeros memset on first pass only)

_CACHE: dict = {}


def _band_weights() -> np.ndarray:
    w = np.zeros((128, MBLK), dtype=ml_dtypes.bfloat16)
    for m in range(MBLK):
        w[m : m + WIN, m] = 1.0
    return w


def _build():
    f32 = mybir.dt.float32
    bf16 = mybir.dt.bfloat16
    add = mybir.AluOpType.add
    sub = mybir.AluOpType.subtract
    nc = bacc.Bacc("TRN2", target_bir_lowering=False, debug=False, num_devices=B)
    x_d = nc.dram_tensor("x", [C, H, W], bf16, kind="ExternalInput")
    w_d = nc.dram_tensor("w", [128, MBLK], bf16, kind="ExternalInput")
    we_d = nc.dram_tensor("we", [8, 16], bf16, kind="ExternalInput")
    o_d = nc.dram_tensor("o", [C, H, W], bf16, kind="ExternalOutput")

    with tile.TileContext(nc) as tc:
        with (
            tc.tile_pool(name="wpool", bufs=1) as wpool,
            tc.tile_pool(name="xpool", bufs=4) as xpool,
            tc.tile_pool(name="xtpool", bufs=2) as xtpool,
            tc.tile_pool(name="upool", bufs=UBUFS) as upool,
            tc.tile_pool(name="utpool", bufs=2) as utpool,
            tc.tile_pool(name="opool", bufs=4) as opool,
            tc.tile_pool(name="otpool", bufs=2) as otpool,
            tc.tile_pool(name="pepool", bufs=2) as pepool,
            tc.tile_pool(name="psum", bufs=4, space="PSUM") as psum,
        ):
            # w on the Scalar ring: issues in parallel with the Sync-ring x
            # loads so the first matmul's weights arrive sooner
            w_t = wpool.tile([128, MBLK], bf16)
            nc.scalar.dma_start(w_t[:], w_d.ap())
            we_t = wpool.tile([8, 16], bf16)
            nc.scalar.dma_start(we_t[:], we_d.ap())

            def vert(x_t, q, m, k):
                """band matmul: x rows -> psum v [m, 1024] (9-row sums)."""
                v_t = psum.tile([MBLK, W], f32, tag="v", bufs=2)
                for n in (0, 512):
                    nc.tensor.matmul(
                        v_t[0:m, n : n + 512],
                        w_t[0:k, 0:m],
                        x_t[0:k, q, n : n + 512],
                        start=True,
                        stop=True,
                    )
                return v_t

            def evict_seg(u_t, g, v_t, m):
                """ACT: both wrap groups (one 2-group op, emitted first so the
                scheduler can't defer it past the next evict) + scaled evict."""
                # wrapL (cols g+9..g+12) <- v[1020..1023],
                # wrapR (cols g+1037..g+1040) <- v[0..3]
                nc.scalar.mul(
                    out=AP(
                        u_t.tensor,
                        u_t.offset + g + WIN,
                        [list(u_t.ap)[0], [W + R, 2], [1, R]],
                    ),
                    in_=AP(
                        v_t.tensor,
                        v_t.offset + W - R,
                        [list(v_t.ap)[0], [-(W - R), 2], [1, R]],
                    ),
                    mul=1.0 / AREA,
                )
                nc.scalar.mul(
                    out=u_t[0:m, g + WIN + R : g + WIN + R + W],
                    in_=v_t[0:m, :],
                    mul=1.0 / AREA,
                )

            def scan(o_t, u_t, m, nseg):
                # out col c of segment q sits at scan index q*SEG + 8 + c
                nc.vector.tensor_tensor_scan(
                    out=o_t[0:m, 0 : nseg * SEG - WIN],
                    data0=u_t[0:m, WIN : nseg * SEG],
                    data1=u_t[0:m, 0 : nseg * SEG - WIN],
                    initial=0.0,
                    op0=add,
                    op1=sub,
                )

            def tail0(c):
                """First unit, optimized for pipeline fill: the load, matmul,
                eviction and scan are split by column halves so the first scan
                only waits for half the data. Segment layout (1075 cols):
                  SEG1 [9z | v 0..511]                     @ 0    (521)
                  SEG2 [9z | v 504..1023 | wrapR v 0..3]   @ 521  (533)
                  SEG3 [9z | wrapL v 1020..23 | v 0..7]    @ 1054 (21)
                """
                r0 = 8 * MBLK - R  # 956
                x_t = xtpool.tile([128, 1, W], bf16, tag=f"xt0_{c}", bufs=1)
                for n in (0, 512):
                    nc.sync.dma_start(
                        x_t[0 : H - r0, 0, n : n + 512],
                        x_d.ap()[c, r0:H, n : n + 512],
                    )
                    nc.sync.dma_start(
                        x_t[H - r0 : KT, 0, n : n + 512],
                        x_d.ap()[c, 0 : KT - (H - r0), n : n + 512],
                    )
                u_t = utpool.tile([MBLK, 1075], bf16, tag=f"ut0_{c}", bufs=1)
                for g in (0, 521, 1054):
                    nc.gpsimd.memset(u_t[0:MT, g : g + WIN], 0.0)
                v_t = psum.tile([MBLK, W], f32, tag="v", bufs=2)
                sc = 1.0 / AREA
                nc.tensor.matmul(
                    v_t[0:MT, 0:512], w_t[0:KT, 0:MT], x_t[0:KT, 0, 0:512],
                    start=True, stop=True)
                nc.tensor.matmul(
                    v_t[0:MT, 512:1024], w_t[0:KT, 0:MT], x_t[0:KT, 0, 512:1024],
                    start=True, stop=True)
                nc.scalar.mul(out=u_t[0:MT, 9:521], in_=v_t[0:MT, 0:512], mul=sc)
                nc.scalar.mul(out=u_t[0:MT, 530:1050], in_=v_t[0:MT, 504:1024], mul=sc)
                nc.scalar.mul(out=u_t[0:MT, 1050:1054], in_=v_t[0:MT, 0:4], mul=sc)
                nc.scalar.mul(out=u_t[0:MT, 1063:1067], in_=v_t[0:MT, 1020:1024], mul=sc)
                nc.scalar.mul(out=u_t[0:MT, 1067:1075], in_=v_t[0:MT, 0:8], mul=sc)
                o_t = otpool.tile([MBLK, 1075], bf16, tag=f"ot0_{c}", bufs=1)
                # scan1 over SEG1: outputs c=4..507 at idx 8..511
                nc.vector.tensor_tensor_scan(
                    out=o_t[0:MT, 0:512], data0=u_t[0:MT, 9:521],
                    data1=u_t[0:MT, 0:512], initial=0.0, op0=add, op1=sub)
                # scan2 over SEG2+SEG3: c=508..1023 at idx 8..523, c=0..3 at 541..544
                nc.vector.tensor_tensor_scan(
                    out=o_t[0:MT, 521:1066], data0=u_t[0:MT, 530:1075],
                    data1=u_t[0:MT, 521:1066], initial=0.0, op0=add, op1=sub)
                nc.gpsimd.dma_start(
                    o_d.ap()[c, 8 * MBLK : H, R : R + 504],
                    o_t[0:MT, 8:512],
                )
                nc.gpsimd.dma_start(
                    o_d.ap()[c, 8 * MBLK : H, 508:1024],
                    o_t[0:MT, 529:1045],
                )
                nc.gpsimd.dma_start(
                    o_d.ap()[c, 8 * MBLK : H, 0:R],
                    o_t[0:MT, 1062:1066],
                )

            def pe_unit(x_t, q, c, j):
                """Horizontal pass on the PE instead of the DVE scan.
                vT windows (x col-slice stationary x w -> transposed 9-row
                sums in PSUM), evict to bf16, band matmul (vT stationary x
                w streaming) -> o[r, c] in row-major orientation; both bands
                are the same w_t. Windows c0=4+120*w (k0=c0-4, all within
                0..1023, PSUM partition base 0); the 8 circular-wrap output
                cols come from a tiny 16-k edge path with we_t weights.
                """
                row0 = (2 * j + q) * MBLK
                sc = 1.0 / AREA
                wins = [(4 + 120 * wd, 120, 120 * wd, 128) for wd in range(8)]
                wins.append((964, 56, 960, 64))  # (c0, width, k0, kn)

                def vt_chunk(tag_tiles, lo, hi):
                    D = psum.tile([128, 480], f32, tag="d", bufs=2)
                    for wd in range(lo, hi):
                        c0, wid, k0, kn = wins[wd]
                        nc.tensor.matmul(
                            D[0:kn, 120 * (wd - lo) : 120 * (wd - lo) + MBLK],
                            x_t[0:128, q, k0 : k0 + kn],
                            w_t[0:128, 0:MBLK],
                            start=True, stop=True)
                    E = pepool.tile([128, 480], bf16, tag=f"e{lo}")
                    nc.scalar.copy(out=E[:, 0 : 120 * (hi - lo)],
                                   in_=D[:, 0 : 120 * (hi - lo)])
                    return E

                E1 = vt_chunk("e0", 0, 4)
                E2 = vt_chunk("e4", 4, 8)
                E3 = vt_chunk("e8", 8, 9)
                # edge vT: k in {1016..1023} and {0..7}
                Ea = psum.tile([128, 480], f32, tag="d", bufs=2)
                nc.tensor.matmul(Ea[0:8, 0:MBLK], x_t[0:128, q, W - 8 : W],
                                 w_t[0:128, 0:MBLK], start=True, stop=True)
                Eea = pepool.tile([8, MBLK], bf16, tag="eea")
                nc.scalar.copy(out=Eea[:], in_=Ea[0:8, 0:MBLK])
                Eb = psum.tile([128, 480], f32, tag="d", bufs=2)
                nc.tensor.matmul(Eb[0:8, 0:MBLK], x_t[0:128, q, 0:8],
                                 w_t[0:128, 0:MBLK], start=True, stop=True)
                Eeb = pepool.tile([8, MBLK], bf16, tag="eeb")
                nc.scalar.copy(out=Eeb[:], in_=Eb[0:8, 0:MBLK])

                o_sb = pepool.tile([MBLK, W], bf16, tag="ope")
                OA = psum.tile([MBLK, 480], f32, tag="oc", bufs=2)
                for wd in range(4):
                    nc.tensor.matmul(
                        OA[0:MBLK, 120 * wd : 120 * wd + 120],
                        E1[:, 120 * wd : 120 * wd + 120],
                        w_t[0:128, 0:120], start=True, stop=True)
                nc.scalar.mul(out=o_sb[:, 4:484], in_=OA[:], mul=sc)
                OB = psum.tile([MBLK, 480], f32, tag="oc", bufs=2)
                for wd in range(4):
                    nc.tensor.matmul(
                        OB[0:MBLK, 120 * wd : 120 * wd + 120],
                        E2[:, 120 * wd : 120 * wd + 120],
                        w_t[0:128, 0:120], start=True, stop=True)
                nc.scalar.mul(out=o_sb[:, 484:964], in_=OB[:], mul=sc)
                OC = psum.tile([MBLK, 480], f32, tag="oc", bufs=2)
                nc.tensor.matmul(OC[0:MBLK, 0:56], E3[0:64, 0:MBLK],
                                 w_t[0:64, 0:56], start=True, stop=True)
                nc.scalar.mul(out=o_sb[:, 964:1020], in_=OC[:, 0:56], mul=sc)
                Oe = psum.tile([MBLK, 480], f32, tag="oc", bufs=2)
                nc.tensor.matmul(Oe[0:MBLK, 0:8], Eea[:], we_t[0:8, 0:8],
                                 start=True, stop=False)
                nc.tensor.matmul(Oe[0:MBLK, 0:8], Eeb[:], we_t[0:8, 8:16],
                                 start=False, stop=True)
                nc.scalar.mul(out=o_sb[:, 1020:1024], in_=Oe[:, 0:4], mul=sc)
                nc.scalar.mul(out=o_sb[:, 0:4], in_=Oe[:, 4:8], mul=sc)
                nc.gpsimd.dma_start(o_d.ap()[c, row0 : row0 + MBLK, :], o_sb[:])

            def pe_load(c, j):
                r0 = 2 * j * MBLK - R
                x_t = xpool.tile([128, 2, W], bf16, tag="xpe", bufs=1)
                nc.sync.dma_start(
                    x_t[:],
                    AP(x_d, c * H * W + r0 * W, [[W, 128], [MBLK * W, 2], [1, W]]),
                )
                return x_t

            def half_pair(c, j, first_rotation):
                """q=0 on the PE path, q=1 via a single-segment scan."""
                r0 = 2 * j * MBLK - R
                x_t = xpool.tile([128, 2, W], bf16, tag="xh", bufs=1)
                nc.sync.dma_start(
                    x_t[:],
                    AP(x_d, c * H * W + r0 * W, [[W, 128], [MBLK * W, 2], [1, W]]),
                )
                pe_unit(x_t, 0, c, j)
                u_t = upool.tile([MBLK, SEG], bf16, tag="u1", bufs=1)
                if first_rotation:
                    nc.gpsimd.memset(u_t[:, 0:WIN], 0.0)
                v_t = vert(x_t, 1, MBLK, 128)
                evict_seg(u_t, 0, v_t, MBLK)
                o_t = opool.tile([MBLK, SEG - WIN], bf16, tag="o1", bufs=1)
                scan(o_t, u_t, MBLK, 1)
                nc.gpsimd.dma_start(
                    o_d.ap()[c, (2 * j + 1) * MBLK : (2 * j + 2) * MBLK, :],
                    o_t[:, 2 * R : 2 * R + W],
                )

            def tail(c, first_rotation):
                r0 = 8 * MBLK - R  # 956
                x_t = xtpool.tile([128, 1, W], bf16, tag="xt")
                nc.sync.dma_start(x_t[0 : H - r0, 0, :], x_d.ap()[c, r0:H, :])
                nc.sync.dma_start(
                    x_t[H - r0 : KT, 0, :], x_d.ap()[c, 0 : KT - (H - r0), :]
                )
                u_t = utpool.tile([MBLK, SEG], bf16, tag="ut")
                if first_rotation:
                    nc.gpsimd.memset(u_t[0:MT, 0:WIN], 0.0)
                v_t = vert(x_t, 0, MT, KT)
                evict_seg(u_t, 0, v_t, MT)
                o_t = otpool.tile([MBLK, SEG - WIN], bf16, tag="ot")
                scan(o_t, u_t, MT, 1)
                if c == 2:
                    # final store: split across two rings so both halves fly
                    # in parallel right before the end-of-kernel barrier
                    h = MT // 2
                    nc.sync.dma_start(
                        o_d.ap()[c, 8 * MBLK : 8 * MBLK + h, :],
                        o_t[0:h, 2 * R : 2 * R + W],
                    )
                    nc.gpsimd.dma_start(
                        o_d.ap()[c, 8 * MBLK + h : H, :],
                        o_t[h:MT, 2 * R : 2 * R + W],
                    )
                else:
                    nc.gpsimd.dma_start(
                        o_d.ap()[c, 8 * MBLK : H, :], o_t[0:MT, 2 * R : 2 * R + W]
                    )

            def pair(c, j, idx):
                r0 = 2 * j * MBLK - R
                x_t = xpool.tile([128, 2, W], bf16, tag="x")
                if j == 0:
                    nc.sync.dma_start(x_t[0:R, 0, :], x_d.ap()[c, H - R : H, :])
                    nc.sync.dma_start(x_t[R:128, 0, :], x_d.ap()[c, 0 : 128 - R, :])
                    nc.sync.dma_start(
                        x_t[:, 1, :], x_d.ap()[c, MBLK - R : MBLK - R + 128, :]
                    )
                else:
                    nc.sync.dma_start(
                        x_t[:],
                        AP(x_d, c * H * W + r0 * W, [[W, 128], [MBLK * W, 2], [1, W]]),
                    )
                u_t = upool.tile([MBLK, 2 * SEG], bf16, tag="u")
                if idx < UBUFS:  # zeros persist across pool rotations
                    nc.gpsimd.memset(u_t[:, 0:WIN], 0.0)
                    nc.gpsimd.memset(u_t[:, SEG : SEG + WIN], 0.0)
                for q in range(2):
                    v_t = vert(x_t, q, MBLK, 128)
                    evict_seg(u_t, SEG * q, v_t, MBLK)
                o_t = opool.tile([MBLK, 2 * SEG - WIN], bf16, tag="o")
                scan(o_t, u_t, MBLK, 2)
                if j == 3:
                    # late stores: one block per ring so the final transfers
                    # fly on two queues in parallel before the end barrier
                    for q, ring in ((0, nc.sync), (1, nc.gpsimd)):
                        ring.dma_start(
                            o_d.ap()[c, (2 * j + q) * MBLK : (2 * j + q + 1) * MBLK, :],
                            o_t[:, SEG * q + 2 * R : SEG * q + 2 * R + W],
                        )
                else:
                    # one 2D store: 240 consecutive output rows, both segments
                    nc.gpsimd.dma_start(
                        AP(
                            o_d,
                            c * H * W + 2 * j * MBLK * W,
                            [[W, MBLK], [MBLK * W, 2], [1, W]],
                        ),
                        AP(
                            o_t.tensor,
                            o_t.offset + 2 * R,
                            [list(o_t.ap)[0], [SEG, 2], [1, W]],
                        ),
                    )

            tail0(0)
            tail(1, True)
            PE1 = (0, 2)   # both units on the PE path
            HP = (2, 2)    # q=0 on PE, q=1 scanned
            dve_seq = [
                (c, j) for j in (1, 0, 2, 3) for c in range(C)
                if (c, j) not in (PE1, HP)
            ]
            pe_x = None
            for i, (c, j) in enumerate(dve_seq):
                pair(c, j, i)
                if i == 2:
                    pe_x = pe_load(*PE1)
                    pe_unit(pe_x, 0, *PE1)
                elif i == 5:
                    pe_unit(pe_x, 1, *PE1)
                elif i == 7:
                    half_pair(*HP, True)
            tail(2, True)
    nc.compile()
    return nc


def _get_nc():
    if "nc" not in _CACHE:
        _CACHE["nc"] = _build()
    return _CACHE["nc"]


def _edge_weights() -> np.ndarray:
    # col 0..7: lhs k in {1016..1023}; col 8..15: lhs k in {0..7}
    # out c2: 0..3 -> c in {1020..1023}, 4..7 -> c in {0..3}
    we = np.zeros((8, 16), dtype=ml_dtypes.bfloat16)
    for kl in range(8):
        for c2 in range(8):
            if kl >= c2:
                we[kl, c2] = 1.0       # wea: k_l >= c2
            if kl <= c2:
                we[kl, 8 + c2] = 1.0   # web: k_l <= c2
    return we


def _prepare_in_maps(tensor: np.ndarray) -> list:
    x = np.asarray(tensor, dtype=np.float32)
    assert x.shape == (B, C, H, W), x.shape
    xb = x.astype(ml_dtypes.bfloat16)
    wmat = _band_weights()
    wedge = _edge_weights()
    return [
        {"x": np.ascontiguousarray(xb[i]), "w": wmat, "we": wedge}
        for i in range(B)
    ]


def kernel(tensor: np.ndarray) -> np.ndarray:
    nc = _get_nc()
    in_maps = _prepare_in_maps(tensor)
    res = run_bass_kernel_spmd(nc, in_maps, core_ids=list(range(B)))
    return np.stack(
        [res.results[i]["o"].astype(np.float32) for i in range(B)], axis=0
    )
